# revision 5
# baseline (speedup 1.0000x reference)
"""Trainium2 Bass kernel for nn_MixingBlock (equivariant sequence-conv mixing
block). Data-parallel over batch: B=8 -> one batch element per NeuronCore.

Math (per batch, per token n, K=3 window, m = n+k-1):
  rhat_k[n]   = normalized displacement coords[m]-coords[n]  (host-precomputed)
  conv scalar: h_s = sum_k [x_s[m], x_v[m].rhat_k] @ Ws_k / sqrt(1152)
  conv vector: h_v[:,d] = sum_k [x_v[m][:,d], x_s[m]*r_k[d], cross(x_v[m],rhat)_d] @ Wv_k / sqrt(1536)
  then equivariant LN -> self-interaction (+residual) -> LN -> block residual -> LN.

Key algebraic folds used on device (all exact up to ~1e-6 eps skew):
  - LN1 scale and LN1-v entirely vanish under LN2 (per-token scale commutes
    through channel matmuls and LN is scale-invariant).
  - LN1-s mean subtraction folds into a rank-1 correction  -mu1 (x) colsum(Wsi'')
    where Wsi'' = I + W_si_s/sqrt(cs).
  - LN2 shift drops under LN3 mean removal; only the LN2 scale survives:
      z0 = x + inv2 * y*,   z = (z0 - mu3) * inv3   (honest LN3)

Layout: channels on partitions, tokens on the free axis. Per-token scalars
(rhat components, LN stats) are broadcast across partitions either on host
(rhat -> 'rb' tensor) or via K=1 rank-1 PE matmuls (LN scales).
All heavy compute in fp16 (PE matmul 1 cyc/row, DVE 2x) with fp32 PSUM
accumulation and fp32 LN row statistics.
"""

import numpy as np

B, N, CS, CV, K = 8, 2048, 256, 128, 3
T = 512
NT = N // T
NPAD = N + 4  # token m lives at padded index m+1; zeros elsewhere
EPS = 1e-6
NW = 38  # packed weight tiles

_CACHE = {}


# ---------------------------------------------------------------------------
# Module build (device program)
# ---------------------------------------------------------------------------

def _build_module():
    import concourse.bass as bass
    import concourse.tile as tile
    import concourse.mybir as mybir
    from contextlib import ExitStack

    fp16 = mybir.dt.float16
    fp32 = mybir.dt.float32
    MULT = mybir.AluOpType.mult
    ADD = mybir.AluOpType.add
    Copy = mybir.ActivationFunctionType.Copy
    Sqrt = mybir.ActivationFunctionType.Sqrt

    nc = bass.Bass()

    xs16 = nc.dram_tensor("xs16", [2, 128, NPAD], fp16, kind="ExternalInput")
    xv16 = nc.dram_tensor("xv16", [3, 128, NPAD], fp16, kind="ExternalInput")
    rb = nc.dram_tensor("rb", [NT, 128, 9, T], fp16, kind="ExternalInput")
    wpack = nc.dram_tensor("wpack", [128, NW * 128], fp16, kind="ExternalInput")
    rows_in = nc.dram_tensor("rows_in", [1, 256], fp16, kind="ExternalInput")
    zs_out = nc.dram_tensor("zs", [2, 128, N], fp32, kind="ExternalOutput")
    zv_out = nc.dram_tensor("zv", [3, 128, N], fp32, kind="ExternalOutput")

    # weight tile indices in wpack
    def j_wss(k, i, o): return k * 4 + i * 2 + o
    def j_wsv(k, o): return 12 + k * 2 + o
    def j_wvv(k): return 18 + k
    def j_wvs(k, i): return 21 + k * 2 + i
    def j_wvc(k): return 27 + k
    def j_wvcn(k): return 30 + k
    def j_wsis(i, o): return 33 + i * 2 + o
    J_WSIV = 37

    CYC = [(1, 2), (2, 0), (0, 1)]  # d -> (d1, d2)

    with ExitStack() as ctx:
        tc = ctx.enter_context(tile.TileContext(nc))
        wt = ctx.enter_context(tc.tile_pool(name="wt", bufs=1))
        xin = ctx.enter_context(tc.tile_pool(name="xin", bufs=2))
        prod = ctx.enter_context(tc.tile_pool(name="prod", bufs=3))
        act = ctx.enter_context(tc.tile_pool(name="act", bufs=2))
        zp = ctx.enter_context(tc.tile_pool(name="zp", bufs=2))
        rowp = ctx.enter_context(tc.tile_pool(name="rowp", bufs=4))
        pbig = ctx.enter_context(tc.tile_pool(name="pbig", bufs=1, space="PSUM"))
        pbank = ctx.enter_context(tc.tile_pool(name="pbank", bufs=3, space="PSUM"))

        wsb = wt.tile([128, NW * 128], fp16)
        nc.sync.dma_start(out=wsb, in_=wpack[:, :])
        negwbar = wt.tile([1, 256], fp16)
        nc.sync.dma_start(out=negwbar, in_=rows_in[:, :])
        onesc = wt.tile([128, 1], fp16)
        nc.vector.memset(onesc, 1.0)
        onesr = wt.tile([1, 128], fp16)
        nc.vector.memset(onesr, 1.0)
        epsr = wt.tile([1, 1], fp32)
        nc.vector.memset(epsr, EPS)

        def W(j):
            return wsb[:, j * 128:(j + 1) * 128]

        Rsqrt = mybir.ActivationFunctionType.Rsqrt

        def act_rsqrt(out, in_, scale):
            eng = nc.scalar
            ins = [eng.lower_ap(in_),
                   eng.lower_ap(epsr.to_broadcast([in_.shape[0], 1])),
                   mybir.ImmediateValue(dtype=fp32, value=scale),
                   mybir.ImmediateValue(dtype=fp32, value=0.0)]
            return eng.add_instruction(mybir.InstActivation(
                name=nc.get_next_instruction_name(), func=Rsqrt,
                ins=ins, outs=[eng.lower_ap(out)]))


        for it in range(NT):
            t0 = it * T

            # ---- input DMAs -------------------------------------------------
            xsA = xin.tile([128, 2, T + 2], fp16, tag="xsA")
            nc.sync.dma_start(out=xsA, in_=xs16[:, :, t0:t0 + T + 2].transpose([1, 0, 2]))
            xsB = xin.tile([128, 2, T + 2], fp16, tag="xsB")
            nc.sync.dma_start(out=xsB, in_=xs16[:, :, t0 + 1:t0 + T + 3].transpose([1, 0, 2]))
            xvA = xin.tile([128, 3, T + 2], fp16, tag="xvA")
            nc.sync.dma_start(out=xvA, in_=xv16[:, :, t0:t0 + T + 2].transpose([1, 0, 2]))
            xvB = xin.tile([128, 3, T + 2], fp16, tag="xvB")
            nc.sync.dma_start(out=xvB, in_=xv16[:, :, t0 + 1:t0 + T + 3].transpose([1, 0, 2]))
            rbt = xin.tile([128, 9, T], fp16, tag="rbt")
            nc.sync.dma_start(out=rbt, in_=rb[it])

            def xs_k(i, k):  # x_s block i shifted by k, [128, T]
                return xsB[:, i, 0:T] if k == 1 else xsA[:, i, k:k + T]

            def xv_k(d, k):  # x_v component d shifted by k, [128, T]
                return xvB[:, d, 0:T] if k == 1 else xvA[:, d, k:k + T]

            # ---- Hadamard products (DVE, fp16 2x) ---------------------------
            # pk[:, b, a, :] = r_k[a] * xv_b(shift k);  gk[:, i, a, :] = r_k[a] * xs_i
            pks, gks = [], []
            for k in range(K):
                rk = rbt[:, 3 * k:3 * k + 3, :]
                pk = prod.tile([128, 3, 3, T], fp16, tag="pk")
                for b in range(3):
                    nc.vector.tensor_mul(
                        pk[:, b, :, :],
                        xv_k(b, k).unsqueeze(1).broadcast_to([128, 3, T]),
                        rk,
                    )
                gk = prod.tile([128, 2, 3, T], fp16, tag="gk")
                for i in range(2):
                    nc.vector.tensor_mul(
                        gk[:, i, :, :],
                        xs_k(i, k).unsqueeze(1).broadcast_to([128, 3, T]),
                        rk,
                    )
                pks.append(pk)
                gks.append(gk)

            # ---- conv matmuls into PSUM ------------------------------------
            conv5 = pbig.tile([128, 5, T], fp32, tag="big5")
            for o in range(2):  # h_s out-blocks
                first = True
                for k in range(K):
                    for i in range(2):
                        nc.tensor.matmul(conv5[:, o, :], W(j_wss(k, i, o)), xs_k(i, k),
                                         start=first, stop=False)
                        first = False
                for k in range(K):
                    for d in range(3):
                        last = (k == K - 1 and d == 2)
                        nc.tensor.matmul(conv5[:, o, :], W(j_wsv(k, o)), pks[k][:, d, d, :],
                                         start=False, stop=last)
            for d in range(3):  # h_v[d]
                d1, d2 = CYC[d]
                first = True
                for k in range(K):
                    nc.tensor.matmul(conv5[:, 2 + d, :], W(j_wvv(k)), xv_k(d, k),
                                     start=first, stop=False)
                    first = False
                    for i in range(2):
                        nc.tensor.matmul(conv5[:, 2 + d, :], W(j_wvs(k, i)), gks[k][:, i, d, :],
                                         start=False, stop=False)
                    nc.tensor.matmul(conv5[:, 2 + d, :], W(j_wvc(k)), pks[k][:, d1, d2, :],
                                     start=False, stop=False)
                    nc.tensor.matmul(conv5[:, 2 + d, :], W(j_wvcn(k)), pks[k][:, d2, d1, :],
                                     start=False, stop=(k == K - 1))

            # ---- LN1 (only mu1 needed) + casts -----------------------------
            hs_sb = act.tile([128, 2, T], fp16, tag="hs")
            nc.scalar.copy(hs_sb, conv5[:, 0:2, :])
            hv_sb = act.tile([128, 3, T], fp16, tag="hv")
            nc.scalar.copy(hv_sb, conv5[:, 2:5, :])

            r_mu1 = pbank.tile([1, T], fp32, tag="bank")
            for i in range(2):
                nc.tensor.matmul(r_mu1, onesc, hs_sb[:, i, :], start=(i == 0), stop=(i == 1))
            mu1row = rowp.tile([1, T], fp16, tag="r16")
            nc.scalar.mul(mu1row, r_mu1, 1.0 / CS)

            # ---- self-interaction: y* = h@W'' - mu1 (x) wbar ----------------
            si5 = pbig.tile([128, 5, T], fp32, tag="big5")
            for o in range(2):
                nc.tensor.matmul(si5[:, o, :], W(j_wsis(0, o)), hs_sb[:, 0, :], start=True, stop=False)
                nc.tensor.matmul(si5[:, o, :], W(j_wsis(1, o)), hs_sb[:, 1, :], start=False, stop=False)
                nc.tensor.matmul(si5[:, o, :], negwbar[0:1, o * 128:(o + 1) * 128], mu1row,
                                 start=False, stop=True)
            for d in range(3):
                nc.tensor.matmul(si5[:, 2 + d, :], W(J_WSIV), hv_sb[:, d, :], start=True, stop=True)

            # ---- LN2 stats --------------------------------------------------
            ys_sb = act.tile([128, 2, T], fp16, tag="ys")
            nc.scalar.copy(ys_sb, si5[:, 0:2, :])
            yv_sb = act.tile([128, 3, T], fp16, tag="yv")
            nc.scalar.copy(yv_sb, si5[:, 2:5, :])
            sq2s = act.tile([128, 2, T], fp16, tag="sq_s")
            nc.vector.tensor_mul(sq2s, ys_sb, ys_sb)
            sq2v = act.tile([128, 3, T], fp16, tag="sq_v")
            nc.vector.tensor_mul(sq2v, yv_sb, yv_sb)

            r_sy = pbank.tile([1, T], fp32, tag="bank")
            for i in range(2):
                nc.tensor.matmul(r_sy, onesc, ys_sb[:, i, :], start=(i == 0), stop=(i == 1))
            r_sy2 = pbank.tile([1, T], fp32, tag="bank")
            for i in range(2):
                nc.tensor.matmul(r_sy2, onesc, sq2s[:, i, :], start=(i == 0), stop=(i == 1))
            r_syv2 = pbank.tile([1, T], fp32, tag="bank")
            for d in range(3):
                nc.tensor.matmul(r_syv2, onesc, sq2v[:, d, :], start=(d == 0), stop=(d == 2))

            # rows: inv2s = rsqrt(var2 + eps), var2 = (Sy2 - (Sy)^2/cs)/cs
            sy = rowp.tile([1, T], fp32, tag="r32")
            nc.scalar.copy(sy, r_sy)
            sqsy = rowp.tile([1, T], fp32, tag="r32")
            nc.vector.tensor_mul(sqsy, sy, sy)
            x2 = rowp.tile([1, T], fp32, tag="r32")
            nc.vector.scalar_tensor_tensor(x2, sqsy, -1.0 / CS, r_sy2, MULT, ADD)
            inv2s = rowp.tile([1, T], fp32, tag="r32")
            act_rsqrt(inv2s, x2, 1.0 / CS)
            inv2s16 = rowp.tile([1, T], fp16, tag="r16")
            nc.scalar.copy(inv2s16, inv2s)

            inv2v = rowp.tile([1, T], fp32, tag="r32")
            act_rsqrt(inv2v, r_syv2, 1.0 / CV)
            inv2v16 = rowp.tile([1, T], fp16, tag="r16")
            nc.scalar.copy(inv2v16, inv2v)

            # broadcasts of inv2
            b2s = pbank.tile([128, T], fp32, tag="bank")
            nc.tensor.matmul(b2s, onesr, inv2s16, start=True, stop=True)
            inv2s_b = act.tile([128, T], fp16, tag="b2s")
            nc.scalar.copy(inv2s_b, b2s)
            b2v = pbank.tile([128, T], fp32, tag="bank")
            nc.tensor.matmul(b2v, onesr, inv2v16, start=True, stop=True)
            inv2v_b = act.tile([128, T], fp16, tag="b2v")
            nc.scalar.copy(inv2v_b, b2v)

            # ---- z0 = x + inv2 * y* ----------------------------------------
            z0s = act.tile([128, 2, T], fp16, tag="z0s")
            nc.vector.tensor_mul(z0s, ys_sb, inv2s_b.unsqueeze(1).broadcast_to([128, 2, T]))
            nc.vector.tensor_add(z0s, z0s, xsB[:, :, 0:T])
            z0v = act.tile([128, 3, T], fp16, tag="z0v")
            nc.vector.tensor_mul(z0v, yv_sb, inv2v_b.unsqueeze(1).broadcast_to([128, 3, T]))
            nc.vector.tensor_add(z0v, z0v, xvB[:, :, 0:T])

            # ---- LN3 stats --------------------------------------------------
            sq3s = act.tile([128, 2, T], fp16, tag="sq_s")
            nc.vector.tensor_mul(sq3s, z0s, z0s)
            sq3v = act.tile([128, 3, T], fp16, tag="sq_v")
            nc.vector.tensor_mul(sq3v, z0v, z0v)

            r_sz = pbank.tile([1, T], fp32, tag="bank")
            for i in range(2):
                nc.tensor.matmul(r_sz, onesc, z0s[:, i, :], start=(i == 0), stop=(i == 1))
            r_sz2 = pbank.tile([1, T], fp32, tag="bank")
            for i in range(2):
                nc.tensor.matmul(r_sz2, onesc, sq3s[:, i, :], start=(i == 0), stop=(i == 1))
            r_szv2 = pbank.tile([1, T], fp32, tag="bank")
            for d in range(3):
                nc.tensor.matmul(r_szv2, onesc, sq3v[:, d, :], start=(d == 0), stop=(d == 2))

            nmu3 = rowp.tile([1, T], fp32, tag="r32")
            nc.scalar.mul(nmu3, r_sz, -1.0 / CS)
            sqm3 = rowp.tile([1, T], fp32, tag="r32")
            nc.vector.tensor_mul(sqm3, nmu3, nmu3)
            x3 = rowp.tile([1, T], fp32, tag="r32")
            nc.vector.scalar_tensor_tensor(x3, sqm3, -float(CS), r_sz2, MULT, ADD)
            inv3s = rowp.tile([1, T], fp32, tag="r32")
            act_rsqrt(inv3s, x3, 1.0 / CS)
            inv3s16 = rowp.tile([1, T], fp16, tag="r16")
            nc.scalar.copy(inv3s16, inv3s)
            c3s16 = rowp.tile([1, T], fp16, tag="r16")
            nc.vector.tensor_mul(c3s16, nmu3, inv3s)

            inv3v = rowp.tile([1, T], fp32, tag="r32")
            act_rsqrt(inv3v, r_szv2, 1.0 / CV)
            inv3v16 = rowp.tile([1, T], fp16, tag="r16")
            nc.scalar.copy(inv3v16, inv3v)

            b3s = pbank.tile([128, T], fp32, tag="bank")
            nc.tensor.matmul(b3s, onesr, inv3s16, start=True, stop=True)
            inv3s_b = act.tile([128, T], fp16, tag="b3s")
            nc.scalar.copy(inv3s_b, b3s)
            bc3 = pbank.tile([128, T], fp32, tag="bank")
            nc.tensor.matmul(bc3, onesr, c3s16, start=True, stop=True)
            c3s_b = act.tile([128, T], fp16, tag="c3s")
            nc.scalar.copy(c3s_b, bc3)
            b3v = pbank.tile([128, T], fp32, tag="bank")
            nc.tensor.matmul(b3v, onesr, inv3v16, start=True, stop=True)
            inv3v_b = act.tile([128, T], fp16, tag="b3v")
            nc.scalar.copy(inv3v_b, b3v)

            # ---- final normalize + output ----------------------------------
            t16 = act.tile([128, 2, T], fp16, tag="t16")
            nc.vector.tensor_mul(t16, z0s, inv3s_b.unsqueeze(1).broadcast_to([128, 2, T]))
            zs_t = zp.tile([128, 2, T], fp32, tag="zs")
            nc.vector.tensor_add(zs_t, t16, c3s_b.unsqueeze(1).broadcast_to([128, 2, T]))
            zv_t = zp.tile([128, 3, T], fp32, tag="zv")
            nc.vector.tensor_mul(zv_t, z0v, inv3v_b.unsqueeze(1).broadcast_to([128, 3, T]))

            nc.sync.dma_start(out=zs_out[:, :, t0:t0 + T].transpose([1, 0, 2]), in_=zs_t)
            nc.sync.dma_start(out=zv_out[:, :, t0:t0 + T].transpose([1, 0, 2]), in_=zv_t)

    from waitfix_inline import split_excess_waits
    split_excess_waits(nc)
    return nc


# waitfix shipped inline so kernel.py is self-contained
import sys as _sys
import types as _types

_wf = _types.ModuleType("waitfix_inline")
_wf_code = '''
import concourse.mybir as mybir
_CTRL_OPS = {"Drain", "NoOp", "EventSemaphore"}
_ZERO_OPS = {"ISA"}
def split_excess_waits(nc, cap_ctrl=1, cap_other=1):
    fn = nc.m.functions[0]
    ctr = [0]
    def mk_ev(engine, waits):
        ev = mybir.InstEventSemaphore(name=f"waitfix_{ctr[0]}", ins=[], outs=[])
        ctr[0] += 1
        ev.engine = engine
        ev.sync_info = mybir.SyncInfo(on_wait=list(waits), on_update=[])
        return ev
    for b in fn.blocks:
        il = b.instructions
        i = 0
        while i < len(il):
            inst = il[i]
            si = inst.sync_info
            if si is None:
                i += 1
                continue
            waits = list(si.on_wait)
            cap = 0 if inst.opcode in _ZERO_OPS else (cap_ctrl if inst.opcode in _CTRL_OPS else cap_other)
            if len(waits) <= cap:
                i += 1
                continue
            keep = waits[:cap]
            excess = waits[cap:]
            inst.sync_info = mybir.SyncInfo(on_wait=keep, on_update=list(si.on_update))
            pos = i
            for j in range(0, len(excess), cap_ctrl):
                ev = mk_ev(inst.engine, excess[j:j + cap_ctrl])
                il.insert(pos, ev)
                pos += 1
                i += 1
            i += 1
'''
exec(_wf_code, _wf.__dict__)
_sys.modules["waitfix_inline"] = _wf


# ---------------------------------------------------------------------------
# Host-side prep
# ---------------------------------------------------------------------------

def _host_prep(x_s, x_v, coords, W_conv_s, W_conv_v, W_si_s, W_si_v):
    f16 = np.float16

    # --- weights ---
    cs_scale = 1.0 / np.sqrt(K * (CS + CV))   # 1/sqrt(1152)
    cv_scale = 1.0 / np.sqrt(K * (CS + 2 * CV))  # 1/sqrt(1536)
    Wcs = (W_conv_s * cs_scale).astype(np.float32)
    Wcv = (W_conv_v * cv_scale).astype(np.float32)
    Wsis = (np.eye(CS, dtype=np.float32) + W_si_s / np.sqrt(CS))
    Wsiv = (np.eye(CV, dtype=np.float32) + W_si_v / np.sqrt(CV))
    negwbar = -Wsis.sum(axis=0)  # [256]

    wpack = np.zeros((128, NW * 128), np.float32)

    def put(j, blk):
        wpack[:, j * 128:(j + 1) * 128] = blk

    for k in range(K):
        base = k * (CS + CV)
        for i in range(2):
            for o in range(2):
                put(k * 4 + i * 2 + o, Wcs[base + i * 128: base + (i + 1) * 128,
                                           o * 128:(o + 1) * 128])
        for o in range(2):
            put(12 + k * 2 + o, Wcs[base + CS: base + CS + CV, o * 128:(o + 1) * 128])
        vbase = k * (CS + 2 * CV)
        put(18 + k, Wcv[vbase: vbase + CV, :])
        for i in range(2):
            put(21 + k * 2 + i, Wcv[vbase + CV + i * 128: vbase + CV + (i + 1) * 128, :])
        put(27 + k, Wcv[vbase + CV + CS: vbase + 2 * CV + CS, :])
        put(30 + k, -Wcv[vbase + CV + CS: vbase + 2 * CV + CS, :])
    for i in range(2):
        for o in range(2):
            put(33 + i * 2 + o, Wsis[i * 128:(i + 1) * 128, o * 128:(o + 1) * 128])
    put(37, Wsiv)

    wpack16 = wpack.astype(f16)
    rows16 = negwbar.reshape(1, 256).astype(f16)

    # --- rhat (host, fp32, matches reference semantics) ---
    idx = np.arange(N)
    R9 = np.empty((B, 9, N), np.float32)
    for k in range(K):
        m = np.clip(idx + k - 1, 0, N - 1)
        d = coords[:, m, :] - coords  # [B,N,3]
        dn = np.linalg.norm(d, axis=-1, keepdims=True)
        rhat = d * np.where(dn > 1e-6, 1.0 / np.maximum(dn, 1e-6), 0.0)
        R9[:, 3 * k:3 * k + 3, :] = rhat.transpose(0, 2, 1)
    R9 = R9.astype(f16)

    in_maps = []
    for b in range(B):
        xs16 = np.zeros((2, 128, NPAD), f16)
        xs16[:, :, 1:N + 1] = x_s[b].T.reshape(2, 128, N)
        xv16 = np.zeros((3, 128, NPAD), f16)
        xv16[:, :, 1:N + 1] = x_v[b].transpose(2, 1, 0)  # [3, 128, N]
        rb = np.empty((NT, 128, 9, T), f16)
        for it in range(NT):
            rb[it] = np.broadcast_to(R9[b][None, :, it * T:(it + 1) * T], (128, 9, T))
        in_maps.append({
            "xs16": xs16,
            "xv16": xv16,
            "rb": rb,
            "wpack": wpack16,
            "rows_in": rows16,
        })
    return in_maps


def _postprocess(results):
    z_s = np.empty((B, N, CS), np.float32)
    z_v = np.empty((B, N, CV, 3), np.float32)
    for b in range(B):
        zs = results[b]["zs"]  # [2, 128, N]
        zv = results[b]["zv"]  # [3, 128, N]
        z_s[b] = zs.reshape(CS, N).T
        z_v[b] = zv.transpose(2, 1, 0)  # [N, 128, 3]
    return z_s, z_v


def kernel(**inputs):
    from concourse.bass_utils import run_bass_kernel_spmd

    if "nc" not in _CACHE:
        _CACHE["nc"] = _build_module()
    nc = _CACHE["nc"]

    in_maps = _host_prep(
        np.asarray(inputs["x_s"], np.float32),
        np.asarray(inputs["x_v"], np.float32),
        np.asarray(inputs["coords"], np.float32),
        np.asarray(inputs["W_conv_s"], np.float32),
        np.asarray(inputs["W_conv_v"], np.float32),
        np.asarray(inputs["W_si_s"], np.float32),
        np.asarray(inputs["W_si_v"], np.float32),
    )
    res = run_bass_kernel_spmd(nc, in_maps, core_ids=list(range(B)))
    return _postprocess(res.results)


# revision 11
# speedup vs baseline: 1.0164x; 1.0164x over previous
"""Trainium2 Bass kernel for nn_MixingBlock (equivariant sequence-conv mixing
block). Data-parallel over batch: B=8 -> one batch element per NeuronCore.

Math (per batch, per token n, K=3 window, m = n+k-1):
  rhat_k[n]   = normalized displacement coords[m]-coords[n]  (host-precomputed)
  conv scalar: h_s = sum_k [x_s[m], x_v[m].rhat_k] @ Ws_k / sqrt(1152)
  conv vector: h_v[:,d] = sum_k [x_v[m][:,d], x_s[m]*r_k[d], cross(x_v[m],rhat)_d] @ Wv_k / sqrt(1536)
  then equivariant LN -> self-interaction (+residual) -> LN -> block residual -> LN.

Key algebraic folds used on device (all exact up to ~1e-6 eps skew):
  - LN1 scale and LN1-v entirely vanish under LN2 (per-token scale commutes
    through channel matmuls and LN is scale-invariant).
  - LN1-s mean subtraction folds into a rank-1 correction  -mu1 (x) colsum(Wsi'')
    where Wsi'' = I + W_si_s/sqrt(cs).
  - LN2 shift drops under LN3 mean removal; only the LN2 scale survives:
      z0 = x + inv2 * y*,   z = (z0 - mu3) * inv3   (honest LN3)

Layout: channels on partitions, tokens on the free axis. Per-token scalars
(rhat components, LN stats) are broadcast across partitions either on host
(rhat -> 'rb' tensor) or via K=1 rank-1 PE matmuls (LN scales).
All heavy compute in fp16 (PE matmul 1 cyc/row, DVE 2x) with fp32 PSUM
accumulation and fp32 LN row statistics.
"""

import numpy as np

B, N, CS, CV, K = 8, 2048, 256, 128, 3
T = 512
NT = N // T
NPAD = N + 4  # token m lives at padded index m+1; zeros elsewhere
EPS = 1e-6
NW = 38  # packed weight tiles

_CACHE = {}


# ---------------------------------------------------------------------------
# Module build (device program)
# ---------------------------------------------------------------------------

def _build_module():
    import concourse.bass as bass
    import concourse.tile as tile
    import concourse.mybir as mybir
    from contextlib import ExitStack

    fp16 = mybir.dt.float16
    fp32 = mybir.dt.float32
    MULT = mybir.AluOpType.mult
    ADD = mybir.AluOpType.add
    Copy = mybir.ActivationFunctionType.Copy
    Sqrt = mybir.ActivationFunctionType.Sqrt

    nc = bass.Bass()

    xs16 = nc.dram_tensor("xs16", [2, 128, NPAD], fp16, kind="ExternalInput")
    xv16 = nc.dram_tensor("xv16", [3, 128, NPAD], fp16, kind="ExternalInput")
    rb = nc.dram_tensor("rb", [NT, 128, 9, T], fp16, kind="ExternalInput")
    wpack = nc.dram_tensor("wpack", [128, NW * 128], fp16, kind="ExternalInput")
    rows_in = nc.dram_tensor("rows_in", [1, 256], fp16, kind="ExternalInput")
    zs_out = nc.dram_tensor("zs", [2, 128, N], fp32, kind="ExternalOutput")
    zv_out = nc.dram_tensor("zv", [3, 128, N], fp32, kind="ExternalOutput")

    # weight tile indices in wpack
    def j_wss(k, i, o): return k * 4 + i * 2 + o
    def j_wsv(k, o): return 12 + k * 2 + o
    def j_wvv(k): return 18 + k
    def j_wvs(k, i): return 21 + k * 2 + i
    def j_wvc(k): return 27 + k
    def j_wvcn(k): return 30 + k
    def j_wsis(i, o): return 33 + i * 2 + o
    J_WSIV = 37

    CYC = [(1, 2), (2, 0), (0, 1)]  # d -> (d1, d2)

    with ExitStack() as ctx:
        tc = ctx.enter_context(tile.TileContext(nc))
        wt = ctx.enter_context(tc.tile_pool(name="wt", bufs=1))
        xin = ctx.enter_context(tc.tile_pool(name="xin", bufs=2))
        prod = ctx.enter_context(tc.tile_pool(name="prod", bufs=3))
        act = ctx.enter_context(tc.tile_pool(name="act", bufs=2))
        zp = ctx.enter_context(tc.tile_pool(name="zp", bufs=2))
        rowp = ctx.enter_context(tc.tile_pool(name="rowp", bufs=4))
        pbig = ctx.enter_context(tc.tile_pool(name="pbig", bufs=1, space="PSUM"))
        pbank = ctx.enter_context(tc.tile_pool(name="pbank", bufs=3, space="PSUM"))

        wsb = wt.tile([128, NW * 128], fp16)
        nc.sync.dma_start(out=wsb, in_=wpack[:, :])
        negwbar = wt.tile([1, 256], fp16)
        nc.sync.dma_start(out=negwbar, in_=rows_in[:, :])
        onesc = wt.tile([128, 1], fp16)
        nc.vector.memset(onesc, 1.0)
        onesr = wt.tile([1, 128], fp16)
        nc.vector.memset(onesr, 1.0)
        epsr = wt.tile([1, 1], fp32)
        nc.vector.memset(epsr, EPS)

        def W(j):
            return wsb[:, j * 128:(j + 1) * 128]

        Rsqrt = mybir.ActivationFunctionType.Rsqrt

        def act_rsqrt(out, in_, scale):
            eng = nc.scalar
            ins = [eng.lower_ap(in_),
                   eng.lower_ap(epsr.to_broadcast([in_.shape[0], 1])),
                   mybir.ImmediateValue(dtype=fp32, value=scale),
                   mybir.ImmediateValue(dtype=fp32, value=0.0)]
            return eng.add_instruction(mybir.InstActivation(
                name=nc.get_next_instruction_name(), func=Rsqrt,
                ins=ins, outs=[eng.lower_ap(out)]))


        def stageA(it):
            t0 = it * T

            # ---- input DMAs -------------------------------------------------
            xsA = xin.tile([128, 2, T + 2], fp16, tag="xsA")
            nc.sync.dma_start(out=xsA, in_=xs16[:, :, t0:t0 + T + 2].transpose([1, 0, 2]))
            xsB = xin.tile([128, 2, T + 2], fp16, tag="xsB")
            nc.sync.dma_start(out=xsB, in_=xs16[:, :, t0 + 1:t0 + T + 3].transpose([1, 0, 2]))
            xvA = xin.tile([128, 3, T + 2], fp16, tag="xvA")
            nc.sync.dma_start(out=xvA, in_=xv16[:, :, t0:t0 + T + 2].transpose([1, 0, 2]))
            xvB = xin.tile([128, 3, T + 2], fp16, tag="xvB")
            nc.sync.dma_start(out=xvB, in_=xv16[:, :, t0 + 1:t0 + T + 3].transpose([1, 0, 2]))
            rbt = xin.tile([128, 9, T], fp16, tag="rbt")
            nc.sync.dma_start(out=rbt, in_=rb[it])

            def xs_k(i, k):  # x_s block i shifted by k, [128, T]
                return xsB[:, i, 0:T] if k == 1 else xsA[:, i, k:k + T]

            def xv_k(d, k):  # x_v component d shifted by k, [128, T]
                return xvB[:, d, 0:T] if k == 1 else xvA[:, d, k:k + T]

            # ---- Hadamard products (DVE, fp16 2x) ---------------------------
            # pk[:, b, a, :] = r_k[a] * xv_b(shift k);  gk[:, i, a, :] = r_k[a] * xs_i
            pks, gks = [], []
            for k in range(K):
                rk = rbt[:, 3 * k:3 * k + 3, :]
                pk = prod.tile([128, 3, 3, T], fp16, tag="pk")
                for b in range(3):
                    nc.vector.tensor_mul(
                        pk[:, b, :, :],
                        xv_k(b, k).unsqueeze(1).broadcast_to([128, 3, T]),
                        rk,
                    )
                gk = prod.tile([128, 2, 3, T], fp16, tag="gk")
                for i in range(2):
                    nc.gpsimd.tensor_mul(
                        gk[:, i, :, :],
                        xs_k(i, k).unsqueeze(1).broadcast_to([128, 3, T]),
                        rk,
                    )
                pks.append(pk)
                gks.append(gk)

            # ---- conv matmuls into PSUM ------------------------------------
            conv5 = pbig.tile([128, 5, T], fp32, tag="big5")
            for o in range(2):  # h_s out-blocks
                first = True
                for k in range(K):
                    for i in range(2):
                        nc.tensor.matmul(conv5[:, o, :], W(j_wss(k, i, o)), xs_k(i, k),
                                         start=first, stop=False)
                        first = False
                for k in range(K):
                    for d in range(3):
                        last = (k == K - 1 and d == 2)
                        nc.tensor.matmul(conv5[:, o, :], W(j_wsv(k, o)), pks[k][:, d, d, :],
                                         start=False, stop=last)
            for d in range(3):  # h_v[d]
                d1, d2 = CYC[d]
                first = True
                for k in range(K):
                    nc.tensor.matmul(conv5[:, 2 + d, :], W(j_wvv(k)), xv_k(d, k),
                                     start=first, stop=False)
                    first = False
                    nc.tensor.matmul(conv5[:, 2 + d, :], W(j_wvc(k)), pks[k][:, d1, d2, :],
                                     start=False, stop=False)
                    nc.tensor.matmul(conv5[:, 2 + d, :], W(j_wvcn(k)), pks[k][:, d2, d1, :],
                                     start=False, stop=False)
                for k in range(K):
                    for i in range(2):
                        nc.tensor.matmul(conv5[:, 2 + d, :], W(j_wvs(k, i)), gks[k][:, i, d, :],
                                         start=False, stop=(k == K - 1 and i == 1))

            # ---- LN1 (only mu1 needed) + casts -----------------------------
            hs_sb = act.tile([128, 2, T], fp16, tag="hs")
            nc.scalar.copy(hs_sb, conv5[:, 0:2, :])
            hv_sb = act.tile([128, 3, T], fp16, tag="hv")
            nc.scalar.copy(hv_sb, conv5[:, 2:5, :])

            r_mu1 = pbank.tile([1, T], fp32, tag="bank")
            for i in range(2):
                nc.tensor.matmul(r_mu1, onesc, hs_sb[:, i, :], start=(i == 0), stop=(i == 1))
            mu1row = rowp.tile([1, T], fp16, tag="r16")
            nc.scalar.mul(mu1row, r_mu1, 1.0 / CS)

            # ---- self-interaction: y* = h@W'' - mu1 (x) wbar ----------------
            si5 = pbig.tile([128, 5, T], fp32, tag="big5")
            for o in range(2):
                nc.tensor.matmul(si5[:, o, :], W(j_wsis(0, o)), hs_sb[:, 0, :], start=True, stop=False)
                nc.tensor.matmul(si5[:, o, :], W(j_wsis(1, o)), hs_sb[:, 1, :], start=False, stop=False)
                nc.tensor.matmul(si5[:, o, :], negwbar[0:1, o * 128:(o + 1) * 128], mu1row,
                                 start=False, stop=True)
            for d in range(3):
                nc.tensor.matmul(si5[:, 2 + d, :], W(J_WSIV), hv_sb[:, d, :], start=True, stop=True)

            ys_sb = act.tile([128, 2, T], fp16, tag="ys")
            nc.scalar.copy(ys_sb, si5[:, 0:2, :])
            yv_sb = act.tile([128, 3, T], fp16, tag="yv")
            nc.scalar.copy(yv_sb, si5[:, 2:5, :])
            return dict(xsB=xsB, xvB=xvB, ys_sb=ys_sb, yv_sb=yv_sb)

        def stageB(it, env):
            t0 = it * T
            xsB, xvB, ys_sb, yv_sb = env['xsB'], env['xvB'], env['ys_sb'], env['yv_sb']
            # ---- LN2 stats --------------------------------------------------
            sq2s = act.tile([128, 2, T], fp16, tag="sq_s")
            nc.vector.tensor_mul(sq2s, ys_sb, ys_sb)
            sq2v = act.tile([128, 3, T], fp16, tag="sq_v")
            nc.vector.tensor_mul(sq2v, yv_sb, yv_sb)

            r_sy = pbank.tile([1, T], fp32, tag="bank")
            for i in range(2):
                nc.tensor.matmul(r_sy, onesc, ys_sb[:, i, :], start=(i == 0), stop=(i == 1))
            r_sy2 = pbank.tile([1, T], fp32, tag="bank")
            for i in range(2):
                nc.tensor.matmul(r_sy2, onesc, sq2s[:, i, :], start=(i == 0), stop=(i == 1))
            r_syv2 = pbank.tile([1, T], fp32, tag="bank")
            for d in range(3):
                nc.tensor.matmul(r_syv2, onesc, sq2v[:, d, :], start=(d == 0), stop=(d == 2))

            # rows: inv2s = rsqrt(var2 + eps), var2 = (Sy2 - (Sy)^2/cs)/cs
            sy = rowp.tile([1, T], fp32, tag="r32")
            nc.scalar.copy(sy, r_sy)
            sqsy = rowp.tile([1, T], fp32, tag="r32")
            nc.vector.tensor_mul(sqsy, sy, sy)
            x2 = rowp.tile([1, T], fp32, tag="r32")
            nc.vector.scalar_tensor_tensor(x2, sqsy, -1.0 / CS, r_sy2, MULT, ADD)
            inv2s = rowp.tile([1, T], fp32, tag="r32")
            act_rsqrt(inv2s, x2, 1.0 / CS)
            inv2s16 = rowp.tile([1, T], fp16, tag="r16")
            nc.scalar.copy(inv2s16, inv2s)

            inv2v = rowp.tile([1, T], fp32, tag="r32")
            act_rsqrt(inv2v, r_syv2, 1.0 / CV)
            inv2v16 = rowp.tile([1, T], fp16, tag="r16")
            nc.scalar.copy(inv2v16, inv2v)

            # broadcasts of inv2
            b2s = pbank.tile([128, T], fp32, tag="bank")
            nc.tensor.matmul(b2s, onesr, inv2s16, start=True, stop=True)
            inv2s_b = act.tile([128, T], fp16, tag="b2s")
            nc.scalar.copy(inv2s_b, b2s)
            b2v = pbank.tile([128, T], fp32, tag="bank")
            nc.tensor.matmul(b2v, onesr, inv2v16, start=True, stop=True)
            inv2v_b = act.tile([128, T], fp16, tag="b2v")
            nc.scalar.copy(inv2v_b, b2v)

            # ---- z0 = x + inv2 * y* ----------------------------------------
            z0s = act.tile([128, 2, T], fp16, tag="z0s")
            nc.vector.tensor_mul(z0s, ys_sb, inv2s_b.unsqueeze(1).broadcast_to([128, 2, T]))
            nc.vector.tensor_add(z0s, z0s, xsB[:, :, 0:T])
            z0v = act.tile([128, 3, T], fp16, tag="z0v")
            nc.vector.tensor_mul(z0v, yv_sb, inv2v_b.unsqueeze(1).broadcast_to([128, 3, T]))
            nc.vector.tensor_add(z0v, z0v, xvB[:, :, 0:T])

            # ---- LN3 stats --------------------------------------------------
            sq3s = act.tile([128, 2, T], fp16, tag="sq_s")
            nc.vector.tensor_mul(sq3s, z0s, z0s)
            sq3v = act.tile([128, 3, T], fp16, tag="sq_v")
            nc.vector.tensor_mul(sq3v, z0v, z0v)

            r_sz = pbank.tile([1, T], fp32, tag="bank")
            for i in range(2):
                nc.tensor.matmul(r_sz, onesc, z0s[:, i, :], start=(i == 0), stop=(i == 1))
            r_sz2 = pbank.tile([1, T], fp32, tag="bank")
            for i in range(2):
                nc.tensor.matmul(r_sz2, onesc, sq3s[:, i, :], start=(i == 0), stop=(i == 1))
            r_szv2 = pbank.tile([1, T], fp32, tag="bank")
            for d in range(3):
                nc.tensor.matmul(r_szv2, onesc, sq3v[:, d, :], start=(d == 0), stop=(d == 2))

            nmu3 = rowp.tile([1, T], fp32, tag="r32")
            nc.scalar.mul(nmu3, r_sz, -1.0 / CS)
            sqm3 = rowp.tile([1, T], fp32, tag="r32")
            nc.vector.tensor_mul(sqm3, nmu3, nmu3)
            x3 = rowp.tile([1, T], fp32, tag="r32")
            nc.vector.scalar_tensor_tensor(x3, sqm3, -float(CS), r_sz2, MULT, ADD)
            inv3s = rowp.tile([1, T], fp32, tag="r32")
            act_rsqrt(inv3s, x3, 1.0 / CS)
            inv3s16 = rowp.tile([1, T], fp16, tag="r16")
            nc.scalar.copy(inv3s16, inv3s)
            c3s16 = rowp.tile([1, T], fp16, tag="r16")
            nc.vector.tensor_mul(c3s16, nmu3, inv3s)

            inv3v = rowp.tile([1, T], fp32, tag="r32")
            act_rsqrt(inv3v, r_szv2, 1.0 / CV)
            inv3v16 = rowp.tile([1, T], fp16, tag="r16")
            nc.scalar.copy(inv3v16, inv3v)

            b3s = pbank.tile([128, T], fp32, tag="bank")
            nc.tensor.matmul(b3s, onesr, inv3s16, start=True, stop=True)
            inv3s_b = act.tile([128, T], fp16, tag="b3s")
            nc.scalar.copy(inv3s_b, b3s)
            bc3 = pbank.tile([128, T], fp32, tag="bank")
            nc.tensor.matmul(bc3, onesr, c3s16, start=True, stop=True)
            c3s_b = act.tile([128, T], fp16, tag="c3s")
            nc.scalar.copy(c3s_b, bc3)
            b3v = pbank.tile([128, T], fp32, tag="bank")
            nc.tensor.matmul(b3v, onesr, inv3v16, start=True, stop=True)
            inv3v_b = act.tile([128, T], fp16, tag="b3v")
            nc.scalar.copy(inv3v_b, b3v)

            # ---- final normalize + output ----------------------------------
            t16 = act.tile([128, 2, T], fp16, tag="t16")
            nc.vector.tensor_mul(t16, z0s, inv3s_b.unsqueeze(1).broadcast_to([128, 2, T]))
            zs_t = zp.tile([128, 2, T], fp32, tag="zs")
            nc.vector.tensor_add(zs_t, t16, c3s_b.unsqueeze(1).broadcast_to([128, 2, T]))
            zv_t = zp.tile([128, 3, T], fp32, tag="zv")
            nc.vector.tensor_mul(zv_t, z0v, inv3v_b.unsqueeze(1).broadcast_to([128, 3, T]))

            nc.sync.dma_start(out=zs_out[:, :, t0:t0 + T].transpose([1, 0, 2]), in_=zs_t)
            nc.sync.dma_start(out=zv_out[:, :, t0:t0 + T].transpose([1, 0, 2]), in_=zv_t)


        for it in range(NT):
            stageB(it, stageA(it))
    from waitfix_inline import split_excess_waits
    split_excess_waits(nc)
    return nc


# waitfix shipped inline so kernel.py is self-contained
import sys as _sys
import types as _types

_wf = _types.ModuleType("waitfix_inline")
_wf_code = '''
import concourse.mybir as mybir
_CTRL_OPS = {"Drain", "NoOp", "EventSemaphore"}
_ZERO_OPS = {"ISA"}
def split_excess_waits(nc, cap_ctrl=1, cap_other=1):
    fn = nc.m.functions[0]
    ctr = [0]
    def mk_ev(engine, waits):
        ev = mybir.InstEventSemaphore(name=f"waitfix_{ctr[0]}", ins=[], outs=[])
        ctr[0] += 1
        ev.engine = engine
        ev.sync_info = mybir.SyncInfo(on_wait=list(waits), on_update=[])
        return ev
    for b in fn.blocks:
        il = b.instructions
        i = 0
        while i < len(il):
            inst = il[i]
            si = inst.sync_info
            if si is None:
                i += 1
                continue
            waits = list(si.on_wait)
            cap = 0 if inst.opcode in _ZERO_OPS else (cap_ctrl if inst.opcode in _CTRL_OPS else cap_other)
            if len(waits) <= cap:
                i += 1
                continue
            keep = waits[:cap]
            excess = waits[cap:]
            inst.sync_info = mybir.SyncInfo(on_wait=keep, on_update=list(si.on_update))
            pos = i
            for j in range(0, len(excess), cap_ctrl):
                ev = mk_ev(inst.engine, excess[j:j + cap_ctrl])
                il.insert(pos, ev)
                pos += 1
                i += 1
            i += 1
'''
exec(_wf_code, _wf.__dict__)
_sys.modules["waitfix_inline"] = _wf


# ---------------------------------------------------------------------------
# Host-side prep
# ---------------------------------------------------------------------------

def _host_prep(x_s, x_v, coords, W_conv_s, W_conv_v, W_si_s, W_si_v):
    f16 = np.float16

    # --- weights ---
    cs_scale = 1.0 / np.sqrt(K * (CS + CV))   # 1/sqrt(1152)
    cv_scale = 1.0 / np.sqrt(K * (CS + 2 * CV))  # 1/sqrt(1536)
    Wcs = (W_conv_s * cs_scale).astype(np.float32)
    Wcv = (W_conv_v * cv_scale).astype(np.float32)
    Wsis = (np.eye(CS, dtype=np.float32) + W_si_s / np.sqrt(CS))
    Wsiv = (np.eye(CV, dtype=np.float32) + W_si_v / np.sqrt(CV))
    negwbar = -Wsis.sum(axis=0)  # [256]

    wpack = np.zeros((128, NW * 128), np.float32)

    def put(j, blk):
        wpack[:, j * 128:(j + 1) * 128] = blk

    for k in range(K):
        base = k * (CS + CV)
        for i in range(2):
            for o in range(2):
                put(k * 4 + i * 2 + o, Wcs[base + i * 128: base + (i + 1) * 128,
                                           o * 128:(o + 1) * 128])
        for o in range(2):
            put(12 + k * 2 + o, Wcs[base + CS: base + CS + CV, o * 128:(o + 1) * 128])
        vbase = k * (CS + 2 * CV)
        put(18 + k, Wcv[vbase: vbase + CV, :])
        for i in range(2):
            put(21 + k * 2 + i, Wcv[vbase + CV + i * 128: vbase + CV + (i + 1) * 128, :])
        put(27 + k, Wcv[vbase + CV + CS: vbase + 2 * CV + CS, :])
        put(30 + k, -Wcv[vbase + CV + CS: vbase + 2 * CV + CS, :])
    for i in range(2):
        for o in range(2):
            put(33 + i * 2 + o, Wsis[i * 128:(i + 1) * 128, o * 128:(o + 1) * 128])
    put(37, Wsiv)

    wpack16 = wpack.astype(f16)
    rows16 = negwbar.reshape(1, 256).astype(f16)

    # --- rhat (host, fp32, matches reference semantics) ---
    idx = np.arange(N)
    R9 = np.empty((B, 9, N), np.float32)
    for k in range(K):
        m = np.clip(idx + k - 1, 0, N - 1)
        d = coords[:, m, :] - coords  # [B,N,3]
        dn = np.linalg.norm(d, axis=-1, keepdims=True)
        rhat = d * np.where(dn > 1e-6, 1.0 / np.maximum(dn, 1e-6), 0.0)
        R9[:, 3 * k:3 * k + 3, :] = rhat.transpose(0, 2, 1)
    R9 = R9.astype(f16)

    in_maps = []
    for b in range(B):
        xs16 = np.zeros((2, 128, NPAD), f16)
        xs16[:, :, 1:N + 1] = x_s[b].T.reshape(2, 128, N)
        xv16 = np.zeros((3, 128, NPAD), f16)
        xv16[:, :, 1:N + 1] = x_v[b].transpose(2, 1, 0)  # [3, 128, N]
        rb = np.empty((NT, 128, 9, T), f16)
        for it in range(NT):
            rb[it] = np.broadcast_to(R9[b][None, :, it * T:(it + 1) * T], (128, 9, T))
        in_maps.append({
            "xs16": xs16,
            "xv16": xv16,
            "rb": rb,
            "wpack": wpack16,
            "rows_in": rows16,
        })
    return in_maps


def _postprocess(results):
    z_s = np.empty((B, N, CS), np.float32)
    z_v = np.empty((B, N, CV, 3), np.float32)
    for b in range(B):
        zs = results[b]["zs"]  # [2, 128, N]
        zv = results[b]["zv"]  # [3, 128, N]
        z_s[b] = zs.reshape(CS, N).T
        z_v[b] = zv.transpose(2, 1, 0)  # [N, 128, 3]
    return z_s, z_v


def kernel(**inputs):
    from concourse.bass_utils import run_bass_kernel_spmd

    if "nc" not in _CACHE:
        _CACHE["nc"] = _build_module()
    nc = _CACHE["nc"]

    in_maps = _host_prep(
        np.asarray(inputs["x_s"], np.float32),
        np.asarray(inputs["x_v"], np.float32),
        np.asarray(inputs["coords"], np.float32),
        np.asarray(inputs["W_conv_s"], np.float32),
        np.asarray(inputs["W_conv_v"], np.float32),
        np.asarray(inputs["W_si_s"], np.float32),
        np.asarray(inputs["W_si_v"], np.float32),
    )
    res = run_bass_kernel_spmd(nc, in_maps, core_ids=list(range(B)))
    return _postprocess(res.results)
